# revision 1
# baseline (speedup 1.0000x reference)
"""BiModal attention kernel for Trainium2 (8 NeuronCores, data-parallel over batch).

Per core (one batch b): x, y: [2048, 128] fp32.
  S = x @ y.T                    (float32r matmuls, [2048, 2048])
  E = exp(S)                     (unshifted; softmax is shift-invariant and
                                  |S| <~ 67 so exp stays in fp32/bf16 range)
  a1 = (E @ y) / rowsum(E) * x
  a2 = (E.T @ x) / colsum(E) * y
  out = concat([a1, a2], -1)     ([2048, 256])

Layout: rows are relabeled r = 16*p + b (p = SBUF partition, b = block index)
so every DRAM transfer is contiguous per partition; the relabeling is applied
consistently to s and t everywhere, so the math is unchanged.

x^T / y^T (d-major, needed as f32r matmul operands) are built without the
tensor engine: split into bf16 hi/lo pairs (DVE/ACT), transpose both with the
DMA xbar, and re-merge hi+lo into f32r on GpSimd. bf16(hi)+bf16(lo) carries
~16 mantissa bits >= f32r's ~12, so S keeps f32r accuracy.

Main loop: two 1024-wide column panels; per (row block i): S matmuls (f32r)
-> exp (ACT, PSUM->SBUF bf16, fused row-sum accum) -> xbar transpose of E
into ET -> DVE column-sum partials. o1T chunks (contract over t) interleave
one panel behind to keep PE dense; o2T chunks + o1T tail + epilogues
(PE retranspose + fused gate on DVE) finish.
"""
import sys

sys.path.insert(0, "/opt/trn_rl_repo")

import os
import numpy as np

import concourse.bass as bass
import concourse.mybir as mybir
import concourse.tile as tile
from concourse.tile_rust import add_dep_helper
from concourse import bacc
from concourse.bass_utils import run_bass_kernel_spmd
from concourse.masks import make_identity

f32 = mybir.dt.float32
f32r = mybir.dt.float32r
bf16 = mybir.dt.bfloat16

B = 8
S = 2048
D = 128
P = 128
NB = S // P          # 16 blocks
NP = 2               # panels
PW = S // NP         # panel width (1024)
PB = PW // P         # blocks per panel (8)

_NC_CACHE = None
LAST_EXEC_NS = None


def _build_program(nc):
    x_d = nc.dram_tensor("x", [S, D], f32, kind="ExternalInput").ap()
    y_d = nc.dram_tensor("y", [S, D], f32, kind="ExternalInput").ap()
    out_d = nc.dram_tensor("out", [S, 2 * D], f32, kind="ExternalOutput").ap()

    # contiguous-per-partition views; row r = 16*p + b
    x_dv = x_d.rearrange("(p b) d -> p b d", p=P)      # [128, 16, 128]
    y_dv = y_d.rearrange("(p b) d -> p b d", p=P)
    out_dv = out_d.rearrange("(p b) c -> p b c", p=P)  # [128, 16, 256]

    Exp = mybir.ActivationFunctionType.Exp
    MUL = mybir.AluOpType.mult
    ADD = mybir.AluOpType.add
    SUBR = mybir.AluOpType.subtract
    AX = mybir.AxisListType.X

    with tile.TileContext(nc) as tc:
        with (
            tc.tile_pool(name="sb", bufs=1) as sb,
            tc.tile_pool(name="tp2", bufs=2) as tp2,
            tc.tile_pool(name="tpf", bufs=2) as tpf,
            tc.tile_pool(name="stg", bufs=6) as stg,
            tc.tile_pool(name="ps", bufs=1, space="PSUM") as ps,
        ):
            # ---- persistent SBUF tensors ----
            y_sb = tpf.tile([P, NB, D], f32, tag="vf32")
            x_sb = tpf.tile([P, NB, D], f32, tag="vf32")
            x_hi = sb.tile([P, NB, D], bf16, tag="x_hi")   # doubles as bf16 x
            y_hi = sb.tile([P, NB, D], bf16, tag="y_hi")   # doubles as bf16 y
            x_lo = sb.tile([P, NB, D], bf16, tag="x_lo")
            y_lo = sb.tile([P, NB, D], bf16, tag="y_lo")
            xT = sb.tile([P, NB, P], f32r, tag="xT")       # [d, sb, sp]
            yT = sb.tile([P, NB, P], f32r, tag="yT")       # [d, tb, tp]
            E = sb.tile([P, NB, S], bf16, tag="E")         # [sp, sb, t-pos]
            ET = sb.tile([P, NB, S], bf16, tag="ET")       # [tp, tb, s-pos]
            oT_pool_a = sb.tile([P, S], f32, tag="oT", name="oT_a")
            o1T_sb = oT_pool_a                             # [d, s-pos]
            ident = sb.tile([P, P], f32, tag="ident")
            l1p = sb.tile([P, 2 * NB], f32, tag="l1p")     # [sp, 2*i+ct]
            l2p = sb.tile([P, NB, NB], f32, tag="l2p")     # [tp, tb, i]
            l1 = sb.tile([P, NB], f32, tag="l1")
            l2 = sb.tile([P, NB], f32, tag="l2")
            r1 = sb.tile([P, NB], f32, tag="r1")
            r2 = sb.tile([P, NB], f32, tag="r2")

            make_identity(nc, ident[:])
            nc.sync.dma_start(y_sb[:, 0:PB], y_dv[:, 0:PB])
            nc.sync.dma_start(x_sb[:], x_dv)
            nc.sync.dma_start(y_sb[:, PB:NB], y_dv[:, PB:NB])

            # ---- prologue: xT/yT via hi/lo bf16 split + xbar transpose ----
            # hi = bf16(v) on ACT; lo = bf16(v - hi) on DVE;
            # xbar: [p, (b d)] -> [d, b, p]; merge hi+lo -> f32r on GpSimd.
            def build_T(v_sb, v_hi, v_lo, vT, name, halves=(0, 1)):
                tT_hi = tp2.tile([P, NB, P], bf16, tag="tT_hi", name=f"th_{name}")
                tT_lo = tp2.tile([P, NB, P], bf16, tag="tT_lo", name=f"tl_{name}")
                for h in halves:
                    sl = slice(h * PB, (h + 1) * PB)
                    nc.scalar.copy(v_hi[:, sl], v_sb[:, sl])
                    nc.vector.tensor_tensor(v_lo[:, sl], v_sb[:, sl],
                                            v_hi[:, sl], op=SUBR)
                    nc.sync.dma_start_transpose(
                        tT_hi[:, sl, :], v_hi[:, sl].rearrange("p b d -> p (b d)"))
                    nc.sync.dma_start_transpose(
                        tT_lo[:, sl, :], v_lo[:, sl].rearrange("p b d -> p (b d)"))
                    # first merge on DVE (fast, idle at head), rest on GpSimd
                    eng = nc.vector if (h == halves[0]) else nc.gpsimd
                    m = eng.tensor_tensor(vT[:, sl, :], tT_hi[:, sl, :],
                                          tT_lo[:, sl, :], op=ADD)
                return m

            m_y0 = build_T(y_sb, y_hi, y_lo, yT, "y", halves=(0,))
            build_T(x_sb, x_hi, x_lo, xT, "x")
            build_T(y_sb, y_hi, y_lo, yT, "y2", halves=(1,))

            # ---- main: panels of 1024 columns ----
            s_rot_a = ps.tile([P, PW], f32, tag="A0", name="s_rot_a")
            s_rot_b = ps.tile([P, PW], f32, tag="A1", name="s_rot_b")
            s_rot = [s_rot_a, s_rot_b]                   # S psum, 2-deep rotation
            o1_ps = ps.tile([P, 4, 512], f32, tag="B")   # o1T accumulator

            def o1_chunk(tb, pin=None):
                for q in range(4):
                    mm = nc.tensor.matmul(o1_ps[:, q, :], y_hi[:, tb, :],
                                          ET[:, tb, q * 512:(q + 1) * 512],
                                          start=(tb == 0), stop=(tb == NB - 1))
                    if q == 0 and pin is not None:
                        add_dep_helper(mm.ins, pin.ins, sync=False,
                                       reason="keep chunk at its emission slot")

            # PE warm-up: dense dummy matmuls so HAM unthrottles before S(0);
            # gated on the first merged data so they run during the x prologue
            yh_f = y_hi[:].rearrange("p b d -> p (b d)")
            for w in range(16):
                wm = nc.tensor.matmul(s_rot[0][:, 0:256], y_hi[:, 0, :],
                                      yh_f[:, 0:256], start=True, stop=True)
                if w == 0:
                    add_dep_helper(wm.ins, m_y0.ins, sync=True,
                                   reason="warmup starts once merges begin")

            yT_f = yT[:].rearrange("p b d -> p (b d)")
            for ct in range(NP):
                c0 = ct * PW
                for i in range(NB):
                    xti = xT[:, i, :]
                    slot = s_rot[i % 2][:]
                    nc.tensor.matmul(slot[:, 0:512], xti,
                                     yT_f[:, c0:c0 + 512], start=True, stop=True)
                    sm = nc.tensor.matmul(slot[:, 512:1024], xti,
                                          yT_f[:, c0 + 512:c0 + 1024],
                                          start=True, stop=True)
                    # interleave one o1T chunk of the previous panel (lagged so
                    # the chunk's transposed inputs are ready when PE reaches it)
                    if ct > 0 and 3 <= i < 3 + PB:
                        o1_chunk((ct - 1) * PB + (i - 3), pin=sm)
                    nc.scalar.activation(E[:, i, c0:c0 + PW], slot, Exp,
                                         accum_out=l1p[:, 2 * i + ct:2 * i + ct + 1])
                    nc.sync.dma_start_transpose(
                        ET[:, ct * PB:(ct + 1) * PB, i * P:(i + 1) * P],
                        E[:, i, c0:c0 + PW])
                    nc.vector.tensor_reduce(
                        l2p[:, ct * PB:(ct + 1) * PB, i],
                        ET[:, ct * PB:(ct + 1) * PB, i * P:(i + 1) * P],
                        axis=AX, op=ADD)

            # ---- normalizers ----
            nc.vector.tensor_reduce(l1[:], l1p[:].rearrange("p (i c) -> p i c", c=2),
                                    axis=AX, op=ADD)
            nc.vector.reciprocal(r1[:], l1[:])
            nc.vector.tensor_reduce(l2[:], l2p[:], axis=AX, op=ADD)
            nc.vector.reciprocal(r2[:], l2[:])

            # ---- final phase: o2T + trailing o1T chunks + epilogue 1 ----
            x_sb2 = tpf.tile([P, NB, D], f32, tag="vf32")
            nc.sync.dma_start(x_sb2[:], x_dv)
            y_sb2 = tpf.tile([P, NB, D], f32, tag="vf32")
            nc.sync.dma_start(y_sb2[:], y_dv)

            o2_ps_a = ps.tile([P, 2, 512], f32, tag="A0")
            o2_ps_b = ps.tile([P, 2, 512], f32, tag="A1")
            o2_q = [o2_ps_a[:, 0, :], o2_ps_a[:, 1, :], o2_ps_b[:, 0, :], o2_ps_b[:, 1, :]]

            e1_ps = None

            def epi1_step(j, pin=None):
                st1 = stg.tile([P, D], f32, tag="st", name=f"st1_{j}")
                tr = nc.tensor.transpose(e1_ps[:, j % 4, 0:P],
                                         o1T_sb[:, j * P:(j + 1) * P], ident[:])
                if pin is not None:
                    add_dep_helper(tr.ins, pin.ins, sync=False,
                                   reason="keep epi1 at its emission slot")
                nc.vector.scalar_tensor_tensor(st1[:], e1_ps[:, j % 4, 0:P],
                                               r1[:, j:j + 1], x_sb2[:, j, :],
                                               op0=MUL, op1=MUL)
                nc.sync.dma_start(out_dv[:, j, 0:D], st1[:])

            LAG = 4
            for i in range(NB):
                for q in range(4):
                    om = nc.tensor.matmul(o2_q[q], x_hi[:, i, :],
                                          E[:, i, q * 512:(q + 1) * 512],
                                          start=(i == 0), stop=(i == NB - 1))
                if LAG <= i < LAG + PB:
                    o1_chunk((NP - 1) * PB + (i - LAG), pin=om)
                if i == LAG + PB - 1:
                    # all o1T chunks issued; drain accumulator and start epi-1
                    nc.scalar.copy(o1T_sb[:, 0:1024],
                                   o1_ps[:, 0:2].rearrange("p a b -> p (a b)"))
                    nc.scalar.copy(o1T_sb[:, 1024:2048],
                                   o1_ps[:, 2:4].rearrange("p a b -> p (a b)"))
                    e1_ps = ps.tile([P, 4, 512], f32, tag="B")
                if i >= LAG + PB:
                    for k in range(4):
                        epi1_step(4 * (i - LAG - PB) + k, pin=om if k == 0 else None)

            o2T_sb = sb.tile([P, S], f32, tag="oT", name="oT_b")
            nc.scalar.copy(o2T_sb[:, 0:1024], o2_ps_a[:].rearrange("p a b -> p (a b)"))
            nc.scalar.copy(o2T_sb[:, 1024:2048], o2_ps_b[:].rearrange("p a b -> p (a b)"))

            # ---- epilogue 2: a2 = o2 * y * r2 (staged into dead E space) ----
            e2_rot = [ps.tile([P, 512], f32, tag="A0", name="e2a"),
                      ps.tile([P, 512], f32, tag="A1", name="e2b")]
            for j in range(NB):
                st2 = stg.tile([P, D], f32, tag="st", name=f"st2_{j}")
                e2t = e2_rot[j % 2]
                nc.tensor.transpose(e2t[:, 0:P],
                                    o2T_sb[:, j * P:(j + 1) * P], ident[:])
                nc.vector.scalar_tensor_tensor(st2[:], e2t[:, 0:P],
                                               r2[:, j:j + 1], y_sb2[:, j, :],
                                               op0=MUL, op1=MUL)
                nc.sync.dma_start(out_dv[:, j, D:2 * D], st2[:])

    nc.compile()
    return nc


def _get_nc():
    global _NC_CACHE
    if _NC_CACHE is None:
        nc = bacc.Bacc("TRN2", target_bir_lowering=False, debug=False,
                       num_devices=B)
        _NC_CACHE = _build_program(nc)
    return _NC_CACHE


def kernel(x, y):
    global LAST_EXEC_NS
    nc = _get_nc()
    x = np.asarray(x, dtype=np.float32)
    y = np.asarray(y, dtype=np.float32)
    in_maps = [
        {"x": np.ascontiguousarray(x[b]), "y": np.ascontiguousarray(y[b])}
        for b in range(B)
    ]
    trace = bool(int(os.environ.get("KERNEL_TRACE", "0")))
    res = run_bass_kernel_spmd(nc, in_maps, list(range(B)), trace=trace)
    LAST_EXEC_NS = res.exec_time_ns
    return np.stack([res.results[b]["out"] for b in range(B)], axis=0)



# revision 3
# speedup vs baseline: 1.1284x; 1.1284x over previous
"""BiModal attention kernel for Trainium2 (8 NeuronCores, data-parallel over batch).

Per core (one batch b): x, y: [2048, 128] fp32.
  S = x @ y.T                    (f32r matmuls, [2048, 2048])
  E = exp(S)                     (unshifted; softmax is shift-invariant and
                                  |S| <~ 67 so exp stays in fp32/bf16 range)
  a1 = (E @ y) / rowsum(E) * x
  a2 = (E.T @ x) / colsum(E) * y
  out = concat([a1, a2], -1)     ([2048, 256])

Layout: rows are relabeled r = 16*p + b (p = SBUF partition, b = block index)
so every DRAM transfer is contiguous per partition; the relabeling is applied
consistently to s and t everywhere, so the math is unchanged.

Structure (vs the earlier hi/lo-split version): xT/yT are built with PE
transposes at the head (exact f32, doubles as HAM warmup), so the first S
matmul issues ~3us in.  Three dense phases keep PE busy end to end:
  panel 0: S(:, 0:1024) + exp + ET xbar issue + o2 cols 0:1024 accum (lag 2)
  panel 1: S(:, 1024:2048) + exp + o2 cols 1024:2048 accum + l2 colsums of
           panel-0 t-blocks (one-shot contiguous DVE reduces) + early a2
           epilogue for t-blocks 0..7
  final:   o1T accumulation (4 psum-bank groups over s-chunks) + remaining
           epilogues, pipelined per group.
Engines: ACT = exp only; Sync = ET transpose issues; GpSimd = output DMA;
DVE = merges/casts/rowsums/colsums/drains/gating muls.
"""
import sys

sys.path.insert(0, "/opt/trn_rl_repo")

import os
import numpy as np

import concourse.bass as bass
import concourse.mybir as mybir
import concourse.tile as tile
from concourse.tile_rust import add_dep_helper
from concourse import bacc
from concourse.bass_utils import run_bass_kernel_spmd
from concourse.masks import make_identity

f32 = mybir.dt.float32
f32r = mybir.dt.float32r
bf16 = mybir.dt.bfloat16

B = 8
S = 2048
D = 128
P = 128
NB = S // P          # 16 blocks
NP = 2               # panels
PW = S // NP         # panel width (1024)
PB = PW // P         # t-blocks per panel (8)

_NC_CACHE = None
LAST_EXEC_NS = None


def _build_program(nc):
    x_d = nc.dram_tensor("x", [S, D], f32, kind="ExternalInput").ap()
    y_d = nc.dram_tensor("y", [S, D], f32, kind="ExternalInput").ap()
    out_d = nc.dram_tensor("out", [S, 2 * D], f32, kind="ExternalOutput").ap()

    # contiguous-per-partition views; row r = 16*p + b
    x_dv = x_d.rearrange("(p b) d -> p b d", p=P)      # [128, 16, 128]
    y_dv = y_d.rearrange("(p b) d -> p b d", p=P)
    out_dv = out_d.rearrange("(p b) c -> p b c", p=P)  # [128, 16, 256]

    Exp = mybir.ActivationFunctionType.Exp
    MUL = mybir.AluOpType.mult
    ADD = mybir.AluOpType.add
    AX = mybir.AxisListType.X

    with tile.TileContext(nc) as tc:
        with (
            tc.tile_pool(name="sb", bufs=1) as sb,
            tc.tile_pool(name="stg", bufs=2) as stg,
            tc.tile_pool(name="ps", bufs=1, space="PSUM") as ps,
        ):
            # ---- persistent SBUF tensors ----
            x_sb = sb.tile([P, NB, D], f32, tag="x_sb")
            y_sb = sb.tile([P, NB, D], f32, tag="y_sb")
            xT = sb.tile([P, NB, P], f32r, tag="xT")       # [d, sb, sp]
            yT = sb.tile([P, NB, P], f32r, tag="yT")       # [d, tb, tp]
            x_hi = sb.tile([P, NB, D], bf16, tag="x_hi")
            y_hi = sb.tile([P, NB, D], bf16, tag="y_hi")
            E = sb.tile([P, NB, S], bf16, tag="E")         # [sp, sb, t]
            ET = sb.tile([P, NB, S], bf16, tag="ET")       # [tp, tb, s]
            o1T = sb.tile([P, S], f32, tag="o1T")          # [d, s]
            o2T = sb.tile([P, S], f32, tag="o2T")          # [d, t]
            ident = sb.tile([P, P], f32, tag="ident")
            l1p = sb.tile([P, NB, 2], f32, tag="l1p")      # [sp, sb, panel]
            l1 = sb.tile([P, NB], f32, tag="l1")
            r1 = sb.tile([P, NB], f32, tag="r1")
            l2 = sb.tile([P, NB], f32, tag="l2")
            r2 = sb.tile([P, NB], f32, tag="r2")

            # PSUM: 8 banks total.
            #   A0/A1 [P,1024] (2 banks each): S psum rotation -> final e-rot
            #   B0/B1 [P,512]: o2 q0/q1 accum -> early-a2 rot -> o1 groups
            #   C0/C1 [P,512]: prologue transpose staging -> o2 q2/q3 accum
            sA = [ps.tile([P, PW], f32, tag="A0", name="sA0"),
                  ps.tile([P, PW], f32, tag="A1", name="sA1")]
            oB = [ps.tile([P, 512], f32, tag="B0", name="oB0"),
                  ps.tile([P, 512], f32, tag="B1", name="oB1")]
            oC = [ps.tile([P, 512], f32, tag="C0", name="oC0"),
                  ps.tile([P, 512], f32, tag="C1", name="oC1")]

            make_identity(nc, ident[:])

            # ---- input DMA (ACT + Sync are the HWDGE issue queues) ----
            nc.scalar.dma_start(y_sb[:, 0:4], y_dv[:, 0:4])
            nc.scalar.dma_start(x_sb[:, 0:4], x_dv[:, 0:4])
            nc.scalar.dma_start(y_sb[:, 4:8], y_dv[:, 4:8])
            nc.scalar.dma_start(x_sb[:, 4:8], x_dv[:, 4:8])
            nc.sync.dma_start(y_sb[:, 8:16], y_dv[:, 8:16])
            nc.sync.dma_start(x_sb[:, 8:16], x_dv[:, 8:16])

            # ---- head: PE transposes of x/y blocks (warmup + exact f32) ----
            ntr = [0]

            def block_T(which, b):
                v_sb, vT = (x_sb, xT) if which == "x" else (y_sb, yT)
                k = ntr[0] % 2
                ntr[0] += 1
                tr = nc.tensor.transpose(oC[k][:, 0:P], v_sb[:, b, :], ident[:])
                nc.vector.tensor_copy(vT[:, b, :], oC[k][:, 0:P])
                return tr

            head_list = [("y", 0), ("y", 1), ("y", 2), ("y", 3), ("x", 0),
                         ("y", 4), ("y", 5), ("y", 6), ("y", 7),
                         ("x", 1), ("x", 2), ("x", 3)]
            for which, b in head_list:
                block_T(which, b)
            rest_list = ([("x", b) for b in range(4, 16)]
                         + [("y", b) for b in range(8, 16)])

            # bf16 stationaries for o1/o2 (cast on DVE)
            nc.vector.tensor_copy(x_hi[:, 0:8], x_sb[:, 0:8])

            yT_f = yT[:].rearrange("p b d -> p (b d)")

            # ---- panel 0: S cols 0:1024, o2 q0/q1 accumulation ----
            for i in range(NB):
                slot = sA[i % 2][:]
                sm0 = nc.tensor.matmul(slot[:, 0:512], xT[:, i, :],
                                       yT_f[:, 0:512], start=True, stop=True)
                sm = nc.tensor.matmul(slot[:, 512:1024], xT[:, i, :],
                                      yT_f[:, 512:1024], start=True, stop=True)
                if i >= 2:
                    # remaining x/y transposes, 2 per iter
                    for _ in range(2):
                        if rest_list:
                            tr = block_T(*rest_list.pop(0))
                            add_dep_helper(tr.ins, sm.ins, sync=False,
                                           reason="keep transpose at slot")
                    j = i - 2
                    for q in range(2):
                        om = nc.tensor.matmul(oB[q][:], x_hi[:, j, :],
                                              E[:, j, q * 512:(q + 1) * 512],
                                              start=(j == 0), stop=(j == NB - 1))
                        if q == 0:
                            add_dep_helper(om.ins, sm.ins, sync=False,
                                           reason="keep o2 at slot")
                nc.scalar.activation(E[:, i, 0:PW], slot, Exp)
                nc.sync.dma_start_transpose(
                    ET[:, 0:PB, i * P:(i + 1) * P], E[:, i, 0:PW])
                nc.vector.tensor_reduce(l1p[:, i, 0:1], E[:, i, 0:PW],
                                        axis=AX, op=ADD)
                if i == 4:
                    nc.vector.tensor_copy(x_hi[:, 8:16], x_sb[:, 8:16])
                if i == 6:
                    nc.vector.tensor_copy(y_hi[:, 0:8], y_sb[:, 0:8])
                if i == 8:
                    nc.vector.tensor_copy(y_hi[:, 8:16], y_sb[:, 8:16])
            # o2 q0/q1 tail (blocks 14, 15)
            for j in (14, 15):
                for q in range(2):
                    nc.tensor.matmul(oB[q][:], x_hi[:, j, :],
                                     E[:, j, q * 512:(q + 1) * 512],
                                     start=False, stop=(j == NB - 1))

            # ---- panel 1: S cols 1024:2048, o2 q2/q3, l2 + early a2 epi ----
            st2g = [None, None, None, None]

            def epi2_step(j, pin=None):
                g = j // 4
                if j % 4 == 0:
                    st2g[g] = stg.tile([P, 4, D], f32, tag="st2",
                                       name=f"st2_{g}")
                bank = sA[j % 2] if j >= 8 else oB[j % 2]
                tr = nc.tensor.transpose(bank[:, 0:P],
                                         o2T[:, j * P:(j + 1) * P], ident[:])
                if pin is not None:
                    add_dep_helper(tr.ins, pin.ins, sync=False,
                                   reason="keep epi2 at slot")
                nc.vector.scalar_tensor_tensor(st2g[g][:, j % 4, :],
                                               bank[:, 0:P], r2[:, j:j + 1],
                                               y_sb[:, j, :], op0=MUL, op1=MUL)
                if j % 4 == 3:
                    nc.gpsimd.dma_start(out_dv[:, g * 4:(g + 1) * 4, D:2 * D],
                                        st2g[g][:])

            for i in range(NB):
                slot = sA[i % 2][:]
                nc.tensor.matmul(slot[:, 0:512], xT[:, i, :],
                                 yT_f[:, 1024:1536], start=True, stop=True)
                sm = nc.tensor.matmul(slot[:, 512:1024], xT[:, i, :],
                                      yT_f[:, 1536:2048], start=True, stop=True)
                if i == 0:
                    # drain o2 cols 0:1024 (frees B banks for early a2 epi)
                    nc.vector.tensor_copy(o2T[:, 0:512], oB[0][:])
                    nc.vector.tensor_copy(o2T[:, 512:1024], oB[1][:])
                if i >= 2:
                    j = i - 2
                    for q in range(2):
                        om = nc.tensor.matmul(oC[q][:], x_hi[:, j, :],
                                              E[:, j, 1024 + q * 512:
                                                 1536 + q * 512],
                                              start=(j == 0), stop=(j == NB - 1))
                        if q == 0:
                            add_dep_helper(om.ins, sm.ins, sync=False,
                                           reason="keep o2 at slot")
                nc.scalar.activation(E[:, i, 1024:2048], slot, Exp)
                nc.sync.dma_start_transpose(
                    ET[:, PB:NB, i * P:(i + 1) * P], E[:, i, 1024:2048])
                nc.vector.tensor_reduce(l1p[:, i, 1:2], E[:, i, 1024:2048],
                                        axis=AX, op=ADD)
                if 1 <= i < 9:
                    tb = i - 1
                    nc.vector.tensor_reduce(l2[:, tb:tb + 1], ET[:, tb, :],
                                            axis=AX, op=ADD)
                if i == 8:
                    nc.vector.reciprocal(r2[:, 0:8], l2[:, 0:8])
                if i >= 8:
                    epi2_step(i - 8, pin=sm)
            for j in (14, 15):
                for q in range(2):
                    nc.tensor.matmul(oC[q][:], x_hi[:, j, :],
                                     E[:, j, 1024 + q * 512:1536 + q * 512],
                                     start=False, stop=(j == NB - 1))

            # ---- final: o1T groups + remaining epilogues ----
            nc.vector.tensor_reduce(l1[:], l1p[:], axis=AX, op=ADD)
            nc.vector.reciprocal(r1[:], l1[:])
            # drain o2 cols 1024:2048
            nc.vector.tensor_copy(o2T[:, 1024:1536], oC[0][:])
            nc.vector.tensor_copy(o2T[:, 1536:2048], oC[1][:])

            st1g = [None, None, None, None]
            erot = [0]

            def epi1_step(j, pin=None):
                g = j // 4
                if j % 4 == 0:
                    st1g[g] = stg.tile([P, 4, D], f32, tag="st1",
                                       name=f"st1_{g}")
                bank = sA[erot[0] % 2]
                erot[0] += 1
                tr = nc.tensor.transpose(bank[:, 0:P],
                                         o1T[:, j * P:(j + 1) * P], ident[:])
                if pin is not None:
                    add_dep_helper(tr.ins, pin.ins, sync=False,
                                   reason="keep epi1 at slot")
                nc.vector.scalar_tensor_tensor(st1g[g][:, j % 4, :],
                                               bank[:, 0:P], r1[:, j:j + 1],
                                               x_sb[:, j, :], op0=MUL, op1=MUL)
                if j % 4 == 3:
                    nc.gpsimd.dma_start(out_dv[:, g * 4:(g + 1) * 4, 0:D],
                                        st1g[g][:])

            def epi2_late(j, pin=None):
                g = j // 4
                if j % 4 == 0:
                    st2g[g] = stg.tile([P, 4, D], f32, tag="st2",
                                       name=f"st2l_{g}")
                bank = sA[erot[0] % 2]
                erot[0] += 1
                tr = nc.tensor.transpose(bank[:, 0:P],
                                         o2T[:, j * P:(j + 1) * P], ident[:])
                if pin is not None:
                    add_dep_helper(tr.ins, pin.ins, sync=False,
                                   reason="keep epi2l at slot")
                nc.vector.scalar_tensor_tensor(st2g[g][:, j % 4, :],
                                               bank[:, 0:P], r2[:, j:j + 1],
                                               y_sb[:, j, :], op0=MUL, op1=MUL)
                if j % 4 == 3:
                    nc.gpsimd.dma_start(out_dv[:, g * 4:(g + 1) * 4, D:2 * D],
                                        st2g[g][:])

            for q in range(4):
                for tb in range(NB):
                    om = nc.tensor.matmul(oB[q % 2][:], y_hi[:, tb, :],
                                          ET[:, tb, q * 512:(q + 1) * 512],
                                          start=(tb == 0), stop=(tb == NB - 1))
                    # interleave previous group's epilogue-1 steps
                    if q >= 1 and tb in (2, 5, 8, 11):
                        jj = 4 * (q - 1) + (2, 5, 8, 11).index(tb)
                        epi1_step(jj, pin=om)
                # drain group q -> o1T cols
                nc.vector.tensor_copy(o1T[:, q * 512:(q + 1) * 512],
                                      oB[q % 2][:])
            for k in range(4):
                epi1_step(12 + k)
            # l2 for t-blocks 8..15 (needs full ET; emitted late to avoid
            # head-of-line blocking on DVE)
            for tb in range(8, 16):
                nc.vector.tensor_reduce(l2[:, tb:tb + 1], ET[:, tb, :],
                                        axis=AX, op=ADD)
            nc.vector.reciprocal(r2[:, 8:16], l2[:, 8:16])
            for j in range(8, 16):
                epi2_late(j)

    nc.compile()
    return nc


def _get_nc():
    global _NC_CACHE
    if _NC_CACHE is None:
        nc = bacc.Bacc("TRN2", target_bir_lowering=False, debug=False,
                       num_devices=B)
        _NC_CACHE = _build_program(nc)
    return _NC_CACHE


def kernel(x, y):
    global LAST_EXEC_NS
    nc = _get_nc()
    x = np.asarray(x, dtype=np.float32)
    y = np.asarray(y, dtype=np.float32)
    in_maps = [
        {"x": np.ascontiguousarray(x[b]), "y": np.ascontiguousarray(y[b])}
        for b in range(B)
    ]
    trace = bool(int(os.environ.get("KERNEL_TRACE", "0")))
    res = run_bass_kernel_spmd(nc, in_maps, list(range(B)), trace=trace)
    LAST_EXEC_NS = res.exec_time_ns
    return np.stack([res.results[b]["out"] for b in range(B)], axis=0)
